# revision 23
# baseline (speedup 1.0000x reference)
"""Trainium2 Bass kernel for nn_Block_9268539425531 (MLA transformer block).

Sharding: 2 batch groups x 4-way TP within each group of 4 cores.
Per core (b = core//4, r = core%4, heads H = [4r, 4r+4)):
  Phase A (own 512 tokens): ln1; dkv = h@w_down -> AllGather {ckv,cq};
           kR = rope(h@w_kr) -> AllGather; qR_all = rope(h@w_qr) for all
           16 heads -> AllToAll (each core receives only its own heads).
  Phase B (all 2048 tokens, own 4 heads): q/k/v from gathered latent;
           all operands stay in SBUF (v in [token, 4*128] layout).
  Phase C: causal attention per head, SBUF-resident, scoresT layout,
           matmul-based partition softmax reductions, depth-2 software
           pipeline so the PE never waits on the exp; per-head AllGather
           of oT.  w_o and first FFN weights prefetched during C/D.
  Phase D: w_o + residual + ln2 on own token slice.
  Phase E: FFN (full hidden dim, own token slice) + residual.
All matmul operands bf16 (same PE rate as fp32r, half the bytes);
accumulation/stats/residuals in fp32.  v-bias folded into b_o on host.
"""
import math
import numpy as np

B, T, C = 2, 2048, 2048
NH = 16
DK = 128
DHR = 64
LAT = 512
P = 128
NT = 512           # tokens per core
CC = C // P        # 16
NCORES = 8
SCALE = 1.0 / math.sqrt(DK)
NEG = -1.0e9
RG = [[0, 1, 2, 3], [4, 5, 6, 7]]

_CACHE = {}


# ---------------------------------------------------------------- program ---
def build_program(repeat=1, nocc=False, stop_after=None):
    from contextlib import ExitStack
    from concourse import bass, bacc, tile, mybir

    dt = mybir.dt
    f32 = dt.float32
    bf16 = dt.bfloat16
    f32r = dt.float32r
    AF = mybir.ActivationFunctionType
    OP = mybir.AluOpType

    nc = bacc.Bacc("TRN2", target_bir_lowering=False, debug=False,
                   num_devices=NCORES)

    def din(name, shape, dtype=bf16):
        return nc.dram_tensor(name, shape, dtype, kind="ExternalInput")

    xT_d = din("xT", [CC, P, NT])
    ln1s_d = din("ln1s", [P, CC], f32)
    ln1b_d = din("ln1b", [P, CC], f32)
    ln2s_d = din("ln2s", [P, CC], f32)
    ln2b_d = din("ln2b", [P, CC], f32)
    wdown_d = din("wdown", [CC, P, 8 * P])
    bdown_d = din("bdown", [P, 8], f32)
    wqr_d = din("wqr", [CC, P, 8 * P])
    bqr_d = din("bqr", [P, 8], f32)
    wkr_d = din("wkr", [CC, P, DHR])
    bkr_d = din("bkr", [DHR, 1], f32)
    r2_d = din("r2", [P, P])
    cosq_d = din("cosq", [8, P, NT])
    sinq_d = din("sinq", [8, P, NT])
    cosk_d = din("cosk", [DHR, NT])
    sink_d = din("sink", [DHR, NT])
    wuk_d = din("wuk", [4, P, 4 * P])
    buk_d = din("buk", [P, 4], f32)
    wuv_d = din("wuv", [4, P, 4 * P])
    wuq_d = din("wuq", [4, P, 4 * P])
    buq_d = din("buq", [P, 4], f32)
    mask_d = din("mask", [4, P, NT])
    ones_r_d = din("ones_r", [P, P])
    wo_d = din("wo", [4, CC, P, 4 * P])
    bo_d = din("bo", [P, CC], f32)
    wff1_d = din("wff1", [CC, 16, P, 4 * P])
    bff1_d = din("bff1", [P, 64], f32)
    wff2_d = din("wff2", [CC, P, 64 * P])
    bff2_d = din("bff2", [P, CC], f32)
    outT_d = nc.dram_tensor("outT", [CC, P, NT], f32, kind="ExternalOutput")

    with tile.TileContext(nc) as tc, ExitStack() as ctx:
        pc = ctx.enter_context(tc.tile_pool(name="const", bufs=1))
        pdram = ctx.enter_context(tc.tile_pool(name="dram", bufs=1, space="DRAM"))

        # ---- small constants resident for the whole kernel
        ones_r = pc.tile([P, P], bf16)
        nc.sync.dma_start(ones_r[:], ones_r_d[:])
        ones_f = pc.tile([P, P], f32r)
        nc.vector.tensor_copy(ones_f[:], ones_r[:])
        r2 = pc.tile([P, P], bf16)
        nc.sync.dma_start(r2[:], r2_d[:])
        ln1s = pc.tile([P, CC], f32)
        nc.sync.dma_start(ln1s[:], ln1s_d[:])
        ln2s = pc.tile([P, CC], f32)
        nc.sync.dma_start(ln2s[:], ln2s_d[:])
        ln1nb = pc.tile([P, CC], f32)      # negated ln biases
        nc.scalar.dma_start(ln1nb[:], ln1b_d[:])
        nc.vector.tensor_scalar_mul(ln1nb[:], ln1nb[:], -1.0)
        ln2nb = pc.tile([P, CC], f32)
        nc.scalar.dma_start(ln2nb[:], ln2b_d[:])
        nc.vector.tensor_scalar_mul(ln2nb[:], ln2nb[:], -1.0)
        bdown = pc.tile([P, 8], f32)
        nc.sync.dma_start(bdown[:], bdown_d[:])
        bqr = pc.tile([P, 8], f32)
        nc.sync.dma_start(bqr[:], bqr_d[:])
        bkr = pc.tile([DHR, 1], f32)
        nc.sync.dma_start(bkr[:], bkr_d[:])
        buk = pc.tile([P, 4], f32)
        nc.sync.dma_start(buk[:], buk_d[:])
        buq = pc.tile([P, 4], f32)
        nc.sync.dma_start(buq[:], buq_d[:])
        bo = pc.tile([P, CC], f32)
        nc.sync.dma_start(bo[:], bo_d[:])
        bff1 = pc.tile([P, 64], f32)
        nc.sync.dma_start(bff1[:], bff1_d[:])
        bff2 = pc.tile([P, CC], f32)
        nc.sync.dma_start(bff2[:], bff2_d[:])
        eps_t = pc.tile([P, 1], f32)
        nc.vector.memset(eps_t[:], 1e-6)

        agin1a = pdram.tile([8, P, NT], bf16)      # 4 ckv + 4 cq
        agin1q = pdram.tile([P, 8 * NT], bf16)     # qR all heads, own tokens
        agin1k = pdram.tile([P, NT], bf16)         # kR own tokens
        agout1ck = pdram.tile([4, 4, P, NT], bf16)  # gathered ckv
        agout1cq = pdram.tile([4, 4, P, NT], bf16)  # gathered cq
        agoutq = pdram.tile([4, P, 8 * NT], bf16)  # gathered qR
        agoutk = pdram.tile([4, P, NT], bf16)      # gathered kR
        agin2 = pdram.tile([4, P, T], bf16)        # own-heads oT
        agout2 = pdram.tile([4, 4, P, T], bf16)    # [head][rank]

        pid = nc.sync.partition_id()
        colo = (pid % 4) * NT
        qoff = (pid % 4) * (2 * NT)

        def layer_norm(src_tiles, pstream, pstat, pool_ps, lns, lnnb,
                       out_slices, nm, ones_t=None, sq_dt=None, stats=None):
            """src [16][P, NT] -> normalized bf16 slices (2 DVE + 2 ACT/ci)."""
            ones_t = ones_r if ones_t is None else ones_t
            sq_dt = bf16 if sq_dt is None else sq_dt
            if stats is not None:
                ps_mean, ps_sq = stats
            else:
                ps_mean = pool_ps.tile([P, NT], f32, name=f"lnpm{nm}")
                ps_sq = pool_ps.tile([P, NT], f32, name=f"lnps{nm}")
                for ci in range(CC):
                    sq = pstream.tile([P, NT], sq_dt, name="lnsq", tag="lnsq")
                    nc.scalar.square(sq[:], src_tiles[ci])
                    nc.tensor.matmul(ps_mean[:], ones_t[:], src_tiles[ci],
                                     start=(ci == 0), stop=(ci == CC - 1),
                                     skip_group_check=True)
                    nc.tensor.matmul(ps_sq[:], ones_t[:], sq[:],
                                     start=(ci == 0), stop=(ci == CC - 1),
                                     skip_group_check=True)
            meanb = pstat.tile([P, NT], f32, name=f"lnmean{nm}")
            nc.vector.tensor_scalar_mul(meanb[:], ps_mean[:], 1.0 / C)
            m2 = pstat.tile([P, NT], f32, name=f"lnm2{nm}")
            nc.vector.tensor_mul(m2[:], meanb[:], meanb[:])
            var = pstat.tile([P, NT], f32, name=f"lnvar{nm}")
            nc.vector.scalar_tensor_tensor(var[:], ps_sq[:], 1.0 / C, m2[:],
                                           OP.mult, OP.subtract)
            std = pstat.tile([P, NT], f32, name=f"lnstd{nm}")
            nc.scalar.activation(std[:], var[:], AF.Sqrt, bias=eps_t[:])
            rstd = pstat.tile([P, NT], f32, name=f"lnrstd{nm}")
            nc.vector.reciprocal(rstd[:], std[:])
            w2 = pstat.tile([P, NT], f32, name=f"lnw2{nm}")
            nc.vector.tensor_mul(w2[:], meanb[:], rstd[:])
            outs = []
            for ci in range(CC):
                # wsb = w2*s - b ; h = (x*s)*rstd - wsb
                wsb = pstream.tile([P, NT], f32, name="lnwsb", tag="lnwsb")
                nc.scalar.activation(wsb[:], w2[:], AF.Identity,
                                     scale=lns[:, ci:ci + 1],
                                     bias=lnnb[:, ci:ci + 1])
                m1 = pstream.tile([P, NT], f32, name="lnm1", tag="lnm1")
                nc.vector.scalar_tensor_tensor(m1[:], src_tiles[ci],
                                               lns[:, ci:ci + 1], rstd[:],
                                               OP.mult, OP.mult)
                h = out_slices[ci]
                nc.vector.tensor_sub(h, m1[:], wsb[:])
                outs.append(h)
            return outs

        _ph = ["A", "B", "C", "D", "E"]
        _upto = len(_ph) if stop_after is None else _ph.index(stop_after) + 1
        _en = set(_ph[:_upto])
        for rep in range(repeat):
            ctx_rep = ExitStack()
            plive = ctx_rep.enter_context(
                tc.tile_pool(name=f"plive{rep}", bufs=1))     # xmid/h2: D..E
            ctx_xt = ExitStack()
            pxt = ctx_xt.enter_context(
                tc.tile_pool(name=f"pxt{rep}", bufs=1))       # xT: A..D
            xTb = pxt.tile([P, CC, NT], bf16, name="xTb")

            # ------------------------------------------------ phase A ----
            with (tc.tile_pool(name=f"pa{rep}", bufs=3) as pa,
                  tc.tile_pool(name=f"pas{rep}", bufs=1) as pas,
                  tc.tile_pool(name=f"pah{rep}", bufs=1) as pah,
                  tc.tile_pool(name=f"paw{rep}", bufs=18) as paw,
                  tc.tile_pool(name=f"pat{rep}", bufs=3) as pat,
                  tc.tile_pool(name=f"pacs{rep}", bufs=1) as pacs,
                  tc.tile_pool(name=f"paps{rep}", bufs=3, space="PSUM") as paps,
                  tc.tile_pool(name=f"papr{rep}", bufs=2, space="PSUM") as papr,
                  tc.tile_pool(name=f"past{rep}", bufs=1, space="PSUM") as pstat):
                cosq = pacs.tile([P, 8, NT], bf16, name="cosq")
                nc.gpsimd.dma_start(cosq[:], cosq_d.transpose([1, 0, 2]))
                sinq = pacs.tile([P, 8, NT], bf16, name="sinq")
                nc.gpsimd.dma_start(sinq[:], sinq_d.transpose([1, 0, 2]))
                cosk = pacs.tile([DHR, NT], bf16, name="cosk")
                nc.sync.dma_start(cosk[:], cosk_d[:])
                sink = pacs.tile([DHR, NT], bf16, name="sink")
                nc.sync.dma_start(sink[:], sink_d[:])
                for ci in range(CC):
                    (nc.sync if ci % 2 == 0 else nc.scalar).dma_start(
                        xTb[:, ci, :], xT_d[ci])
                xT = [xTb[:, ci, :] for ci in range(CC)]
                hb = pah.tile([P, CC, NT], bf16, name="hb")
                hts = layer_norm(xT, pa, pas, pstat, ln1s, ln1nb,
                                 [hb[:, ci, :] for ci in range(CC)], f"h{rep}")
                wd_tiles = []
                for ci in range(CC):
                    w = paw.tile([P, 8 * P], bf16, name="wdt", tag="wdt")
                    nc.gpsimd.dma_start(w[:], wdown_d[ci])
                    wd_tiles.append(w)
                dkvb = pah.tile([P, 8, NT], bf16, name="dkvb")
                for mi in range(8):
                    ps = paps.tile([P, NT], f32, name="psdkv", tag="psdkv")
                    for ci in range(CC):
                        nc.tensor.matmul(ps[:], wd_tiles[ci][:, mi * P:(mi + 1) * P],
                                         hts[ci],
                                         start=(ci == 0), stop=(ci == CC - 1))
                    nc.vector.tensor_scalar_add(dkvb[:, mi, :], ps[:],
                                                bdown[:, mi:mi + 1])
                    if mi == 3:
                        nc.sync.dma_start(agin1a[0:4].transpose([1, 0, 2]),
                                          dkvb[:, 0:4, :])
                        if nocc:
                            nc.sync.dma_start(agout1ck[0], agin1a[0:4])
                        else:
                            nc.gpsimd.collective_compute(
                                "AllGather", mybir.AluOpType.bypass,
                                replica_groups=RG,
                                ins=[agin1a[0:4].opt()],
                                outs=[agout1ck.opt()])
                nc.sync.dma_start(agin1a[4:8].transpose([1, 0, 2]),
                                  dkvb[:, 4:8, :])
                if nocc:
                    nc.sync.dma_start(agout1cq[0], agin1a[4:8])
                else:
                    nc.gpsimd.collective_compute(
                        "AllGather", mybir.AluOpType.bypass, replica_groups=RG,
                        ins=[agin1a[4:8].opt()], outs=[agout1cq.opt()])

                # kR on own tokens (small; its gather flies during qR)
                wk_tiles = []
                for ci in range(CC):
                    w = paw.tile([P, DHR], bf16, name="wkt", tag="wkt")
                    nc.gpsimd.dma_start(w[:], wkr_d[ci])
                    wk_tiles.append(w)
                psk = paps.tile([DHR, NT], f32, name="pskr", tag="psdkv")
                for ci in range(CC):
                    nc.tensor.matmul(psk[:], wk_tiles[ci][:], hts[ci],
                                     start=(ci == 0), stop=(ci == CC - 1))
                prek = pat.tile([DHR, NT], bf16, name="krpre", tag="krpre")
                nc.scalar.activation(prek[:], psk[:], AF.Identity, bias=bkr[:])
                rotk = papr.tile([DHR, NT], f32, name="psrotk", tag="psrot")
                nc.tensor.matmul(rotk[:], r2[0:DHR, 0:DHR], prek[:],
                                 start=True, stop=True)
                tmpk = pat.tile([DHR, NT], f32, name="rtmpk", tag="rtmp")
                nc.vector.tensor_mul(tmpk[:], rotk[:], sink[:])
                tmp2k = pat.tile([DHR, NT], f32, name="rtmp2k", tag="rtmp2")
                nc.vector.tensor_mul(tmp2k[:], prek[:], cosk[:])
                krr = pat.tile([DHR, NT], bf16, name="krr", tag="krr")
                nc.vector.tensor_add(krr[:], tmp2k[:], tmpk[:])
                nc.sync.dma_start(agin1k[0:DHR, :], krr[:])
                if nocc:
                    nc.scalar.dma_start(agoutk[0], agin1k[:])
                else:
                    nc.gpsimd.collective_compute(
                        "AllGather", mybir.AluOpType.bypass, replica_groups=RG,
                        ins=[agin1k.opt()], outs=[agoutk.opt()])

                # qR for all 16 heads on own tokens, rope applied, pipelined
                wq_tiles = []
                for ci in range(CC):
                    w = paw.tile([P, 8 * P], bf16, name="wqt", tag="wdt")
                    nc.gpsimd.dma_start(w[:], wqr_d[ci])
                    wq_tiles.append(w)
                qrb = pah.tile([P, 8, NT], bf16, name="qrb")
                pres = {}

                def rot_emit(j):
                    rot = papr.tile([P, NT], f32, name="psrot", tag="psrot")
                    nc.tensor.matmul(rot[:], r2[:], pres[j][:],
                                     start=True, stop=True,
                                     skip_group_check=True)
                    tmp = pat.tile([P, NT], f32, name="rtmp", tag="rtmp")
                    nc.vector.tensor_mul(tmp[:], rot[:], sinq[:, j, :])
                    tmp2 = pat.tile([P, NT], f32, name="rtmp2", tag="rtmp2")
                    nc.vector.tensor_mul(tmp2[:], pres[j][:], cosq[:, j, :])
                    nc.vector.tensor_add(qrb[:, j, :], tmp2[:], tmp[:])

                for mt in range(8):
                    ps = paps.tile([P, NT], f32, name="psqr", tag="psdkv")
                    for ci in range(CC):
                        nc.tensor.matmul(ps[:], wq_tiles[ci][:, mt * P:(mt + 1) * P],
                                         hts[ci], start=(ci == 0),
                                         stop=(ci == CC - 1),
                                         skip_group_check=True)
                    pre = pat.tile([P, NT], bf16, name="qrpre", tag="qrpre")
                    nc.scalar.activation(pre[:], ps[:], AF.Identity,
                                         bias=bqr[:, mt:mt + 1])
                    pres[mt] = pre
                    if mt > 0:
                        rot_emit(mt - 1)
                rot_emit(7)
                nc.sync.dma_start(agin1q[:], qrb[:])
                if nocc:
                    nc.scalar.dma_start(agoutq[0], agin1q[:])
                else:
                    nc.gpsimd.collective_compute(
                        "AllGather", mybir.AluOpType.bypass, replica_groups=RG,
                        ins=[agin1q.opt()], outs=[agoutq.opt()])

            # ------------------------------------------------ phase B ----
            if "B" not in _en:
                ctx_xt.close()
                ctx_rep.close()
                continue
            ctx_bc = ExitStack()
            pqkv = ctx_bc.enter_context(
                tc.tile_pool(name=f"pqkv{rep}", bufs=1))      # q/k/v: B..C
            qT4 = [pqkv.tile([P, T], bf16, name=f"qT4_{m}") for m in range(4)]
            kT4 = [pqkv.tile([P, T], bf16, name=f"kT4_{m}") for m in range(4)]
            vtb = pqkv.tile([P, CC, 4 * P], bf16, name="vtb")
            with (tc.tile_pool(name=f"pbw{rep}", bufs=1) as pbw,
                  tc.tile_pool(name=f"pbs{rep}", bufs=10) as pbs,
                  tc.tile_pool(name=f"pbps{rep}", bufs=3, space="PSUM") as pbps):
                wuq_sb, wuk_sb, wuv_sb = [], [], []
                for lc in range(4):
                    w = pbw.tile([P, 4 * P], bf16, name=f"wuq{lc}")
                    nc.gpsimd.dma_start(w[:], wuq_d[lc])
                    wuq_sb.append(w)
                    w = pbw.tile([P, 4 * P], bf16, name=f"wuk{lc}")
                    nc.gpsimd.dma_start(w[:], wuk_d[lc])
                    wuk_sb.append(w)
                    w = pbw.tile([P, 4 * P], bf16, name=f"wuv{lc}")
                    nc.gpsimd.dma_start(w[:], wuv_d[lc])
                    wuv_sb.append(w)
                for nt in range(4):
                    nts = slice(nt * NT, (nt + 1) * NT)
                    ckv, cq = [], []
                    for lc in range(4):
                        t = pbs.tile([P, NT], bf16, name="ckvc", tag="ckvc")
                        (nc.sync if lc % 2 == 0 else nc.scalar).dma_start(
                            t[:], agout1ck[nt, lc])
                        ckv.append(t)
                        t = pbs.tile([P, NT], bf16, name="cqc", tag="cqc")
                        (nc.sync if lc % 2 == 0 else nc.scalar).dma_start(
                            t[:], agout1cq[nt, lc])
                        cq.append(t)
                    for mt in range(4):
                        ps = pbps.tile([P, NT], f32, name="psq", tag="psq")
                        for lc in range(4):
                            nc.tensor.matmul(
                                ps[:], wuq_sb[lc][:, mt * P:(mt + 1) * P],
                                cq[lc], start=(lc == 0), stop=(lc == 3))
                        nc.vector.tensor_scalar_add(qT4[mt][:, nts], ps[:],
                                                    buq[:, mt:mt + 1])
                        ps = pbps.tile([P, NT], f32, name="psk", tag="psq")
                        for lc in range(4):
                            nc.tensor.matmul(
                                ps[:], wuk_sb[lc][:, mt * P:(mt + 1) * P],
                                ckv[lc], start=(lc == 0), stop=(lc == 3))
                        nc.vector.tensor_scalar_add(kT4[mt][:, nts], ps[:],
                                                    buk[:, mt:mt + 1])
                    for ttl in range(4):
                        tt = 4 * nt + ttl
                        ps = pbps.tile([P, 4 * P], f32, name="psv", tag="psq")
                        for lc in range(4):
                            nc.tensor.matmul(
                                ps[:], ckv[lc][:, ttl * P:(ttl + 1) * P],
                                wuv_sb[lc][:], start=(lc == 0), stop=(lc == 3))
                        nc.scalar.activation(vtb[:, tt, :], ps[:], AF.Identity)

            # ------------------------------------------------ phase C ----
            if "C" not in _en:
                ctx_bc.close()
                ctx_xt.close()
                ctx_rep.close()
                continue
            ctx_cd = ExitStack()
            pEw = ctx_cd.enter_context(
                tc.tile_pool(name=f"pEw{rep}", bufs=24, side="right"))
            ctx_dw = ExitStack()
            pDw = ctx_dw.enter_context(
                tc.tile_pool(name=f"pDw{rep}", bufs=36, side="right"))
            wo_sb = []

            def load_wo_group(mig):
                for kt in range(CC):
                    w = pDw.tile([P, 4 * P], bf16, name="wo", tag="wo")
                    nc.gpsimd.dma_start(w[:], wo_d[mig, kt])
                    wo_sb.append(w)

            if "D" in _en:
                load_wo_group(0)
                load_wo_group(1)
            with (tc.tile_pool(name=f"pch{rep}", bufs=2) as phd,
                  tc.tile_pool(name=f"pcm{rep}", bufs=1) as pcm,
                  tc.tile_pool(name=f"pce{rep}", bufs=6) as pex,
                  tc.tile_pool(name=f"pco{rep}", bufs=3) as pot,
                  tc.tile_pool(name=f"pcps{rep}", bufs=4, space="PSUM") as pcsc,
                  tc.tile_pool(name=f"pcpo{rep}", bufs=2, space="PSUM") as pcso,
                  tc.tile_pool(name=f"pcpm{rep}", bufs=2, space="PSUM") as pcss):
                masks = pcm.tile([P, 4, NT], bf16, name="masks")
                nc.sync.dma_start(masks[:], mask_d.transpose([1, 0, 2]))
                kRh = pcm.tile([DHR, T], bf16, name="kRh")
                for nt in range(4):
                    nc.sync.dma_start(kRh[:, nt * NT:(nt + 1) * NT],
                                      agoutk[nt][0:DHR, :])
                qRhs = {}

                def load_qrh(m):
                    t = phd.tile([DHR, T], bf16, name="qRh", tag="qRh")
                    off = DHR * (m % 2)
                    for nt in range(4):
                        nc.sync.dma_start(
                            t[:, nt * NT:(nt + 1) * NT],
                            agoutq[nt][off:off + DHR,
                                       bass.ds(qoff + (m // 2) * NT, NT)])
                    qRhs[m] = t

                load_qrh(0)
                for m in range(4):
                    if m < 3:
                        load_qrh(m + 1)
                    qRh = qRhs.pop(m)
                    steps = [(qi, ki) for qi in range(4)
                             for ki in range(4 * qi + 4)]
                    exs = {}
                    psos, psss = {}, {}

                    def score_emit(qi, ki):
                        qs = slice(qi * NT, (qi + 1) * NT)
                        ks = slice(ki * P, (ki + 1) * P)
                        psc = pcsc.tile([P, NT], f32, name="psc", tag="psc")
                        nc.tensor.matmul(psc[:], kT4[m][:, ks],
                                         qT4[m][:, qs], start=True,
                                         stop=False, skip_group_check=True)
                        nc.tensor.matmul(psc[:], kRh[:, ks], qRh[:, qs],
                                         start=False, stop=True,
                                         skip_group_check=True)
                        if ki >= 4 * qi:
                            nc.vector.tensor_add(psc[:], psc[:],
                                                 masks[:, ki - 4 * qi, :])
                        ex = pex.tile([P, NT], bf16, name="ex", tag="ex")
                        nc.scalar.activation(ex[:], psc[:], AF.Exp,
                                             scale=SCALE)
                        exs[(qi, ki)] = ex

                    def accum_emit(qi, ki):
                        nki = 4 * qi + 4
                        if ki == 0:
                            psos[qi] = pcso.tile([P, NT], f32, name="pso",
                                                 tag="pso")
                            psss[qi] = pcss.tile([P, NT], f32, name="pss",
                                                 tag="pss")
                        ex = exs.pop((qi, ki))
                        nc.tensor.matmul(psos[qi][:],
                                         vtb[:, ki, m * P:(m + 1) * P],
                                         ex[:], start=(ki == 0),
                                         stop=(ki == nki - 1),
                                         skip_group_check=True)
                        nc.tensor.matmul(psss[qi][:], ones_r[:], ex[:],
                                         start=(ki == 0),
                                         stop=(ki == nki - 1),
                                         skip_group_check=True)
                        if ki == nki - 1:
                            qs = slice(qi * NT, (qi + 1) * NT)
                            rec = pot.tile([P, NT], f32, name="rec",
                                           tag="rec")
                            nc.vector.reciprocal(rec[:], psss.pop(qi)[:])
                            otb = pot.tile([P, NT], bf16, name="otb",
                                           tag="otb")
                            nc.vector.tensor_mul(otb[:], psos.pop(qi)[:],
                                                 rec[:])
                            nc.sync.dma_start(agin2[m][:, qs], otb[:])
                            if qi == 3:
                                if nocc:
                                    nc.sync.dma_start(agout2[m, 0],
                                                      agin2[m])
                                else:
                                    nc.gpsimd.collective_compute(
                                        "AllGather", mybir.AluOpType.bypass,
                                        replica_groups=RG,
                                        ins=[agin2[m].opt()],
                                        outs=[agout2[m].opt()])

                    score_emit(*steps[0])
                    score_emit(*steps[1])
                    for j in range(len(steps)):
                        if j + 2 < len(steps):
                            score_emit(*steps[j + 2])
                        accum_emit(*steps[j])
            ctx_bc.close()

            # ------------------------------------------------ phase D ----
            if "D" not in _en:
                ctx_dw.close()
                ctx_xt.close()
                ctx_cd.close()
                ctx_rep.close()
                continue
            xmid = [plive.tile([P, NT], f32r, name=f"xmid{mi}")
                    for mi in range(CC)]
            h2b = plive.tile([P, CC, NT], bf16, name="h2b")
            with (tc.tile_pool(name=f"pdo{rep}", bufs=1) as pdo,
                  tc.tile_pool(name=f"pdt{rep}", bufs=4) as pdt,
                  tc.tile_pool(name=f"pds{rep}", bufs=1) as pds,
                  tc.tile_pool(name=f"pdps{rep}", bufs=3, space="PSUM") as pdps,
                  tc.tile_pool(name=f"pdst{rep}", bufs=1, space="PSUM") as pdst):
                # prefetch first FFN weight groups during D
                wf1_pre = []
                if "E" in _en:
                    for mtg in range(1):
                        grp = []
                        for ci in range(CC):
                            w = pEw.tile([P, 4 * P], bf16, name="wf1",
                                         tag="wf1")
                            nc.gpsimd.dma_start(w[:], wff1_d[ci, mtg])
                            grp.append(w)
                        wf1_pre.append(grp)
                otb_ = pdo.tile([P, CC, NT], bf16, name="otb_")
                for m4 in range(4):
                    for rk in range(4):
                        nc.sync.dma_start(
                            otb_[:, 4 * rk + m4, :],
                            agout2[m4, rk][:, bass.ds(colo, NT)])
                for mig in range(4):
                    if mig + 2 < 4:
                        load_wo_group(mig + 2)
                    for ml in range(4):
                        mi = mig * 4 + ml
                        ps = pdps.tile([P, NT], f32, name="pswo", tag="pswo")
                        for kt in range(CC):
                            nc.tensor.matmul(
                                ps[:],
                                wo_sb[mig * CC + kt][:, ml * P:(ml + 1) * P],
                                otb_[:, kt, :], start=(kt == 0),
                                stop=(kt == CC - 1))
                        nc.vector.scalar_tensor_tensor(
                            xmid[mi][:], ps[:], bo[:, mi:mi + 1],
                            xTb[:, mi, :], OP.add, OP.add)
                h2 = layer_norm([t[:] for t in xmid], pdt, pds, pdst,
                                ln2s, ln2nb,
                                [h2b[:, ci, :] for ci in range(CC)],
                                f"g{rep}", ones_t=ones_f, sq_dt=f32r)
            ctx_dw.close()
            ctx_xt.close()

            # ------------------------------------------------ phase E ----
            if "E" not in _en:
                ctx_cd.close()
                ctx_rep.close()
                continue
            with (tc.tile_pool(name=f"pew2{rep}", bufs=2) as pew2,
                  tc.tile_pool(name=f"peg{rep}", bufs=1) as peg,
                  tc.tile_pool(name=f"pet{rep}", bufs=3) as pet,
                  tc.tile_pool(name=f"peps{rep}", bufs=3, space="PSUM") as peps,
                  tc.tile_pool(name=f"pep2{rep}", bufs=2, space="PSUM") as pep2):
                gtb = peg.tile([P, 64, NT], bf16, name="gtb")
                for mtg in range(16):
                    if mtg < 1:
                        wts = wf1_pre[mtg]
                    else:
                        wts = []
                        for ci in range(CC):
                            w = pEw.tile([P, 4 * P], bf16, name="wf1",
                                         tag="wf1")
                            (nc.gpsimd if ci % 2 == 0 else nc.scalar).dma_start(
                                w[:], wff1_d[ci, mtg])
                            wts.append(w)
                    for ml in range(4):
                        mt = mtg * 4 + ml
                        ps = peps.tile([P, NT], f32, name="psf1", tag="psf1")
                        for ci in range(CC):
                            nc.tensor.matmul(
                                ps[:], wts[ci][:, ml * P:(ml + 1) * P],
                                h2[ci],
                                start=(ci == 0), stop=(ci == CC - 1))
                        nc.scalar.activation(gtb[:, mt, :], ps[:],
                                             AF.Gelu_apprx_tanh,
                                             bias=bff1[:, mt:mt + 1])
                for mi in range(CC):
                    w2 = pew2.tile([P, 64 * P], bf16, name="wf2", tag="wf2")
                    nc.sync.dma_start(w2[:], wff2_d[mi])
                    ps2 = pep2.tile([P, NT], f32, name="psf2", tag="psf2")
                    for hl in range(64):
                        nc.tensor.matmul(
                            ps2[:], w2[:, hl * P:(hl + 1) * P],
                            gtb[:, hl, :],
                            start=(hl == 0), stop=(hl == 63))
                    ob = pet.tile([P, NT], f32, name="outb", tag="outb")
                    nc.vector.scalar_tensor_tensor(
                        ob[:], ps2[:], bff2[:, mi:mi + 1],
                        xmid[mi][:], OP.add, OP.add)
                    nc.sync.dma_start(outT_d[mi], ob[:])
            ctx_cd.close()
            ctx_rep.close()

    nc.compile()
    return nc


# ------------------------------------------------------------------ host ---
def _rope_tables(r):
    """cos/sin for qR (all 16 heads, own 512 tokens) and kR (own tokens)."""
    t = np.arange(NT * r, NT * (r + 1), dtype=np.float64) + 1.0
    j = np.arange(NH * DHR)
    theta = 10000.0 ** (-2.0 * (j // 2) / 1024.0)
    ang = t[None, :] * theta[:, None]             # [1024, NT]
    cosq = np.cos(ang).reshape(8, P, NT)
    sinq = np.sin(ang).reshape(8, P, NT)
    l = np.arange(DHR)
    thk = 10000.0 ** (-2.0 * (l // 2) / 64.0)
    angk = t[None, :] * thk[:, None]              # [64, NT]
    cosk = np.cos(angk)
    sink = np.sin(angk)
    f = np.float32
    return (np.ascontiguousarray(cosq, f), np.ascontiguousarray(sinq, f),
            np.ascontiguousarray(cosk, f), np.ascontiguousarray(sink, f))


def _shared_consts():
    r2 = np.zeros((P, P), np.float32)
    for i in range(64):
        r2[2 * i + 1, 2 * i] = -1.0
        r2[2 * i, 2 * i + 1] = 1.0
    mask = np.zeros((4, P, NT), np.float32)
    kl = np.arange(P)[:, None]
    ql = np.arange(NT)[None, :]
    for j in range(4):
        mask[j] = np.where(P * j + kl > ql, NEG, 0.0)
    ones = np.ones((P, P), np.float32)
    return r2, mask, ones


_F32_NAMES = {"ln1s", "ln1b", "ln2s", "ln2b", "bdown", "bqr", "bkr",
              "buk", "buq", "bo", "bff1", "bff2"}


def prepare_in_maps(inputs):
    import ml_dtypes
    bf = ml_dtypes.bfloat16
    f = np.float32
    g = {k: np.asarray(v, f) for k, v in inputs.items()}
    x = g["x"]
    r2, mask, ones = _shared_consts()

    wdown_t = np.ascontiguousarray(g["w_down"].reshape(CC, P, 8 * P))
    bdown_t = np.ascontiguousarray(g["b_down"].reshape(8, P).T)
    wkr_t = np.ascontiguousarray(g["w_kr"].reshape(CC, P, DHR))
    bkr_t = np.ascontiguousarray(g["b_kr"].reshape(DHR, 1))
    wo_t = np.ascontiguousarray(
        g["w_o"].reshape(CC, P, 4, 4 * P).transpose(2, 0, 1, 3))
    bo_eff = g["b_o"] + g["b_ukv"][C:] @ g["w_o"]   # fold v-bias into b_o
    bo_t = np.ascontiguousarray(bo_eff.reshape(CC, P).T)
    wff1_t = np.ascontiguousarray(
        g["w_ff1"].reshape(CC, P, 16, 4 * P).transpose(0, 2, 1, 3))
    bff1_t = np.ascontiguousarray(g["b_ff1"].reshape(64, P).T)
    wff2_t = np.ascontiguousarray(
        g["w_ff2"].reshape(64, P, CC, P).transpose(2, 1, 0, 3)
        .reshape(CC, P, 64 * P))
    bff2_t = np.ascontiguousarray(g["b_ff2"].reshape(CC, P).T)
    ln1s_t = np.ascontiguousarray(g["ln1_scale"].reshape(CC, P).T)
    ln1b_t = np.ascontiguousarray(g["ln1_bias"].reshape(CC, P).T)
    ln2s_t = np.ascontiguousarray(g["ln2_scale"].reshape(CC, P).T)
    ln2b_t = np.ascontiguousarray(g["ln2_bias"].reshape(CC, P).T)

    in_maps = []
    for c in range(NCORES):
        b, r = divmod(c, 4)
        cosq, sinq, cosk, sink = _rope_tables(r)
        xs = x[b, NT * r:NT * (r + 1), :].T                      # [C, NT]
        xT_t = np.ascontiguousarray(xs.reshape(CC, P, NT))
        wuk_c = g["w_ukv"][:, 512 * r:512 * (r + 1)]
        wuv_c = g["w_ukv"][:, C + 512 * r:C + 512 * (r + 1)]
        wuq_c = g["w_uq"][:, 512 * r:512 * (r + 1)]
        m = {
            "xT": xT_t,
            "ln1s": ln1s_t, "ln1b": ln1b_t, "ln2s": ln2s_t, "ln2b": ln2b_t,
            "wdown": wdown_t, "bdown": bdown_t,
            "wqr": np.ascontiguousarray(g["w_qr"].reshape(CC, P, 8 * P)),
            "bqr": np.ascontiguousarray(g["b_qr"].reshape(8, P).T),
            "wkr": wkr_t, "bkr": bkr_t,
            "r2": r2,
            "cosq": cosq, "sinq": sinq, "cosk": cosk, "sink": sink,
            "wuk": np.ascontiguousarray(wuk_c.reshape(4, P, 4 * P)),
            "buk": np.ascontiguousarray(
                g["b_ukv"][512 * r:512 * (r + 1)].reshape(4, P).T),
            "wuv": np.ascontiguousarray(wuv_c.reshape(4, P, 4 * P)),
            "wuq": np.ascontiguousarray(wuq_c.reshape(4, P, 4 * P)),
            "buq": np.ascontiguousarray(
                g["b_uq"][512 * r:512 * (r + 1)].reshape(4, P).T),
            "mask": mask, "ones_r": ones,
            "wo": wo_t, "bo": bo_t,
            "wff1": wff1_t, "bff1": bff1_t,
            "wff2": wff2_t, "bff2": bff2_t,
        }
        for k in m:
            m[k] = np.ascontiguousarray(m[k], f if k in _F32_NAMES else bf)
        in_maps.append(m)
    return in_maps


def assemble_output(results):
    out = np.zeros((B, T, C), np.float32)
    for c in range(NCORES):
        b, r = divmod(c, 4)
        o = results[c]["outT"].reshape(C, NT)
        out[b, NT * r:NT * (r + 1), :] = o.T
    return out


def kernel(**inputs):
    from concourse import bass_utils
    nc = _CACHE.get("nc")
    if nc is None:
        nc = build_program(repeat=1)
        _CACHE["nc"] = nc
    in_maps = prepare_in_maps(inputs)
    res = bass_utils.run_bass_kernel_spmd(nc, in_maps,
                                          core_ids=list(range(NCORES)))
    return assemble_output(res.results)
